# revision 1
# baseline (speedup 1.0000x reference)
"""Trainium2 Bass kernel for nn_AttentionConv2D (sparse_attention).

The reference reduces (pos-never-incremented bug: only im2col slot 0 carries
data, and the Ve slots 1..8 are exactly zero) to, per pixel i (flat h*64+w):

    att0(i) = x_i^T G x_s(i) + u0^T x_i + r^T x_s(i) + cp0      G = s Wq^T Wk
    a_p(i)  = u_p^T x_i + cp_p                                  p = 1..8
    w0      = softmax([att0, a_1..a_8])[0]   (zeroed at w=W-1 / h=H-1)
    out     = (Wv x_s(i) + bv) * w0          x_s(i) = x at pixel i+65 (0-pad)

Sharding: one image (256 x 64 x 64) per NeuronCore, 8 cores data-parallel.

Per-core layout is PIXEL-major: 32 tiles of 128 pixels on partitions.
Per tile: stationary = x c-major slices (shifted / unshifted); one fused
matmul pair produces [V | y] in a single PSUM bank; a scalar_tensor_tensor
with accum_out computes the x.y channel dot (tensor_tensor_reduce wedges
the device); V escapes PSUM immediately via an ACT copy (no w0 dependency,
keeps the bank rotation and the HAM clock warm); the softmax runs on
[128, 4x9] logit banks shared by 4 tiles; V*w0 runs on the otherwise-idle
gpsimd engine from SBUF. The x for the dot is loaded fp8e4 (pixel-major
second copy); V/y GEMMs stay bf16 (fp8 there fails the 2e-2 gate).
The +bv term (bv x w0 outer product) is applied on the host.
"""

import os
import sys

import numpy as np

for _p in ("/opt/trn_rl_repo",):
    if _p not in sys.path:
        sys.path.append(_p)

import concourse.bass as bass
import concourse.tile as tile
from concourse import bacc, mybir
from concourse import bass_utils

F32 = mybir.dt.float32
BF16 = mybir.dt.bfloat16
F8 = mybir.dt.float8e4
AF = mybir.ActivationFunctionType
ALU = mybir.AluOpType
AX = mybir.AxisListType

B, C, H, W = 8, 256, 64, 64
HW = H * W                # 4096
A = 256
SCALE = A ** -0.5
SHIFT = W + 1             # 65
NT = 32                   # pixel tiles (128 px each)
GS = 4                    # tiles per softmax group
NG = NT // GS             # 8 groups
XCOLS = HW + 68           # padded c-major x columns
NCORES = 8
WARMUP = int(os.environ.get("KERNEL_WARMUP", "26"))
NO_CP = bool(os.environ.get("KERNEL_NO_CP"))
NO_Z = bool(os.environ.get("KERNEL_NO_Z"))
NO_SCALEAP = bool(os.environ.get("KERNEL_NO_SCALEAP"))
NO_TTRACC = bool(os.environ.get("KERNEL_NO_TTRACC"))
STAGE = int(os.environ.get("KERNEL_STAGE", "4"))
XPM_ACT = bool(os.environ.get("KERNEL_XPM_ACT"))
J3DVE = bool(os.environ.get("KERNEL_J3DVE"))
PS35 = bool(os.environ.get("KERNEL_PS35"))
DEEP = os.environ.get("KERNEL_DEEP", "1") not in ("", "0")
DEEPER = bool(os.environ.get("KERNEL_DEEPER"))
XPMF = bool(os.environ.get("KERNEL_XPMF"))
B0SPLIT = bool(os.environ.get("KERNEL_B0SPLIT"))

_CACHE = {}
LAST_RESULTS = None


def _build():
    nc = bacc.Bacc("TRN2", target_bir_lowering=False, debug=False)

    xcm_d = nc.dram_tensor("xcm", [128, 2, XCOLS], BF16, kind="ExternalInput").ap()
    xpm_d = nc.dram_tensor("xpm", [128, NT, C], F8, kind="ExternalInput").ap()
    # packed constants: cols 0:512 wm, 512:521 u9, 521:522 rr,
    # 522:558 cp36 (rows 0:2 = the two partition-chunks), 558:560 msk-f32-as-2
    wpk_d = nc.dram_tensor("wpk", [128, 1120], BF16, kind="ExternalInput").ap()
    out_d = nc.dram_tensor("out", [128, NT, C], BF16, kind="ExternalOutput").ap()
    w0_d = nc.dram_tensor("w0o", [128, NT], F32, kind="ExternalOutput").ap()

    with tile.TileContext(nc) as tc:
        with (
            tc.tile_pool(name="const", bufs=1) as const,
            tc.tile_pool(name="grp", bufs=(8 if DEEPER else 6 if DEEP else 4)) as grp,
            tc.tile_pool(name="outp", bufs=(9 if DEEPER else 7 if DEEP else 5)) as outp,
            tc.tile_pool(name="vsbp", bufs=(8 if DEEPER else 6 if DEEP else 4)) as vsbp,
            tc.tile_pool(name="psVY", bufs=(5 if PS35 else 6), space="PSUM") as psVY,
            tc.tile_pool(name="psS", bufs=(3 if PS35 else 2), space="PSUM") as psS,
        ):
            # ---- persistent inputs ----
            xcm2_sb = const.tile([128, 2, XCOLS], BF16, name="xcm2", tag="xcm2")
            xcm_sb = [xcm2_sb[:, k, :] for k in range(2)]
            xpm_sb = const.tile([128, NT, C], F8, name="xpm", tag="xpm")
            wpk2_sb = const.tile([128, 1120], BF16, name="wpk2", tag="wpk2")
            wpk_sb = [wpk2_sb[:, k * 560:(k + 1) * 560] for k in range(2)]
            wm_sb = [wpk_sb[k][:, 0:512] for k in range(2)]
            u_sb = [wpk_sb[k][:, 512:521] for k in range(2)]
            r_sb = [wpk_sb[k][:, 521:522] for k in range(2)]
            cp_sb = wpk_sb[0][0:1, 522:522 + GS * 9]
            mask_sb = wpk_sb[0][:, 558:559]
            ones_sb = const.tile([1, 128], BF16, name="ones", tag="ones")
            w0all = const.tile([128, NT], F32, name="w0all", tag="w0all")
            t0all = const.tile([128, NT], F32, name="t0all", tag="t0all")
            scr = const.tile([128, C], BF16, name="scr", tag="scr")

            nc.sync.dma_start(wpk2_sb[:], wpk_d[:])
            nc.gpsimd.memset(ones_sb[:], 1.0)
            zro_sb = const.tile([128, 1], F32, name="zro", tag="zro")
            nc.vector.memset(zro_sb[:], 0.0)
            # warm the ACT table + any const plumbing before the DMA queue fills
            nc.scalar.activation(scr[0:1, 0:1], zro_sb[0:1, :], AF.Exp,
                                 bias=zro_sb[0:1, :])

            XB = 1041
            nc.sync.dma_start(xcm2_sb[:, :, 0:XB], xcm_d[:, :, 0:XB])
            nc.sync.dma_start(
                xpm_sb[:, 0:2 * GS, :], xpm_d[:, 0:2 * GS, :])
            nc.sync.dma_start(xcm2_sb[:, :, XB:2 * XB], xcm_d[:, :, XB:2 * XB])
            nc.sync.dma_start(
                xpm_sb[:, 2 * GS:4 * GS, :], xpm_d[:, 2 * GS:4 * GS, :])
            nc.sync.dma_start(xcm2_sb[:, :, 2 * XB:XCOLS], xcm_d[:, :, 2 * XB:XCOLS])
            nc.sync.dma_start(
                xpm_sb[:, 4 * GS:NT, :], xpm_d[:, 4 * GS:NT, :])

            # ---- PE warm-up: matmuls on memset data, independent of DMAs ----
            if WARMUP:
                wu_sb = const.tile([128, 128], BF16, name="wu_sb", tag="wu_sb")
                nc.gpsimd.memset(wu_sb[:], 0.0)
                wu_ps = psVY.tile([128, 512], F32, name="wu", tag="vy")
                # MM#0 gets its own bytes: the lint-reader below depends only
                # on it, so the DVE queue is not held behind the whole warm-up
                nc.tensor.matmul(
                    wu_ps[:, 0:16], wu_sb[:], wu_sb[:, 0:16],
                    start=True, stop=True,
                )
                nc.vector.tensor_scalar_add(scr[0:1, 0:1], wu_ps[0:1, 0:1], 0.0)
                for i in range(WARMUP - 1):
                    nc.tensor.matmul(
                        wu_ps[:, 128:256], wu_sb[:], wu_sb[:],
                        start=True, stop=True,
                    )

            state = {}

            def front(g):
                s3 = None
                if STAGE >= 2:
                    s3 = psS.tile([128, GS * 9], F32, name=f"s{g}", tag="s")
                vsb = vsbp.tile([128, GS, C], BF16, name=f"v{g}", tag="v")
                vys = []
                for j in range(GS):
                    t = g * GS + j
                    p0 = t * 128
                    vy = psVY.tile([128, 512], F32, name=f"vy{t}", tag="vy")
                    vys.append(vy)
                    xt = [xcm_sb[k][:, 1 + p0:1 + p0 + 128] for k in range(2)]
                    xs = [xcm_sb[k][:, 1 + p0 + SHIFT:1 + p0 + SHIFT + 128]
                          for k in range(2)]
                    if STAGE >= 2:
                        if j == 0 and not NO_CP:
                            # cp_p broadcast initializes the whole logit bank
                            # (start=True; everything after accumulates)
                            nc.tensor.matmul(
                                s3[:, 0:GS * 9], ones_sb[:], cp_sb,
                                start=True, stop=False, skip_group_check=True,
                            )
                        # a_rest logits: s3[:, j*9+p] += u_p^T x
                        nc.tensor.matmul(
                            s3[:, j * 9:(j + 1) * 9], xt[0], u_sb[0],
                            start=(j == 0 and NO_CP), stop=False,
                            skip_group_check=True,
                        )
                        nc.tensor.matmul(
                            s3[:, j * 9:(j + 1) * 9], xt[1], u_sb[1],
                            start=False, stop=False, skip_group_check=True,
                        )
                    # fused [V | y] GEMM: moving = [Wv^T | s*Wk^T Wq]
                    nc.tensor.matmul(vy[:], xs[0], wm_sb[0], start=True, stop=False)
                    if STAGE >= 2 and not NO_Z:
                        # + r^T x_sh into slot 0 (same stationary as VY mm)
                        nc.tensor.matmul(
                            s3[:, j * 9:j * 9 + 1], xs[0], r_sb[0],
                            start=False, stop=False, skip_group_check=True,
                        )
                    nc.tensor.matmul(vy[:], xs[1], wm_sb[1], start=False, stop=True)
                    if STAGE >= 2 and not NO_Z:
                        nc.tensor.matmul(
                            s3[:, j * 9:j * 9 + 1], xs[1], r_sb[1],
                            start=False, stop=(j == GS - 1), skip_group_check=True,
                        )
                    # V leaves PSUM immediately (no w0 dependency) so the
                    # bank recycles without waiting on the softmax chain
                    nc.scalar.activation(
                        vsb[:, j, :], vy[:, 0:256], AF.Identity,
                        bias=zro_sb[:],
                    )
                    if STAGE < 2 or STAGE < 3:
                        continue
                    # att0 channel dot: t0all[:, t] = sum_c xpm * y
                    nc.vector.scalar_tensor_tensor(
                        scr[:], vy[:, 256:512], 1.0, xpm_sb[:, t, :],
                        ALU.mult, ALU.mult, accum_out=t0all[:, t:t + 1],
                    )
                if STAGE >= 3:
                    # s3[:, j*9] += t0 for the whole group
                    nc.vector.tensor_tensor(
                        s3[:, 0:GS * 9:9], s3[:, 0:GS * 9:9],
                        t0all[:, g * GS:(g + 1) * GS], ALU.add,
                    )
                state[g] = (s3, vsb)

            def mid(g):
                # emitted right after front(g): exp ahead of older V-copies in
                # the ACT queue, so the s-bank frees early for group g+2
                s3, vsb = state.pop(g)
                ex = None
                if STAGE >= 2:
                    ex = grp.tile([128, GS, 9], BF16, name=f"ex{g}", tag="ex")
                    nc.scalar.activation(ex[:], s3[:, 0:GS * 9], AF.Exp,
                                         bias=zro_sb[:])
                state[g] = (ex, vsb)

            def back_dve(g):
                # softmax scalar chain; deps (exp_g) resolved an iteration ago
                ex, vsb = state.pop(g)
                d4 = None
                if STAGE >= 2:
                    d4 = grp.tile([128, GS], F32, name=f"d4{g}", tag="d4")
                    nc.vector.tensor_reduce(d4[:], ex[:], axis=AX.X, op=ALU.add)
                if STAGE >= 4:
                    rd4 = grp.tile([128, GS], F32, name=f"rd4{g}", tag="rd4")
                    nc.vector.reciprocal_approx_fast(rd4[:], d4[:])
                    # w0 = e0 * mask * (1/D)
                    nc.vector.scalar_tensor_tensor(
                        w0all[:, g * GS:(g + 1) * GS], ex[:, :, 0], mask_sb,
                        rd4[:], ALU.mult, ALU.mult,
                    )
                    if g == NG - 1:
                        # h = H-1 boundary: zero the last 64 pixels (tile 31)
                        nc.gpsimd.memset(w0all[64:128, NT - 1:NT], 0.0)
                elif STAGE >= 2:
                    nc.sync.dma_start(w0_d[:, g * GS:(g + 1) * GS], d4[:])
                state[g] = vsb

            def back_gp(g):
                # V * w0 on the (otherwise idle) gpsimd engine, all-SBUF
                vsb = state.pop(g)
                out_sb = outp.tile([128, GS, C], BF16, name=f"o{g}", tag="o")
                last = g == NG - 1
                for j in range(GS):
                    t = g * GS + j
                    if STAGE < 4:
                        nc.gpsimd.tensor_copy(out_sb[:, j, :], vsb[:, j, :])
                    elif (last and j in (1, 3)) or (J3DVE and j == GS - 1):
                        # on DVE (2x bf16) to shorten the group tail
                        nc.vector.tensor_scalar_mul(
                            out_sb[:, j, :], vsb[:, j, :], w0all[:, t:t + 1],
                        )
                    elif last and j == 2:
                        nc.scalar.activation(
                            out_sb[:, j, :], vsb[:, j, :], AF.Identity,
                            bias=zro_sb[:], scale=w0all[:, t:t + 1],
                        )
                    else:
                        nc.gpsimd.tensor_tensor(
                            out_sb[:, j, :], vsb[:, j, :],
                            w0all[:, t:t + 1].broadcast_to((128, C)), ALU.mult,
                        )
                if last:
                    # ship the first half while the second half computes
                    nc.sync.dma_start(
                        out_d[:, g * GS:g * GS + 2, :], out_sb[:, 0:2, :])
                    nc.sync.dma_start(
                        out_d[:, g * GS + 2:(g + 1) * GS, :], out_sb[:, 2:4, :])
                else:
                    nc.sync.dma_start(
                        out_d[:, g * GS:(g + 1) * GS, :], out_sb[:])

            for g in range(NG):
                if g >= 1:
                    back_dve(g - 1)
                front(g)
                if g >= 1:
                    back_gp(g - 1)
                mid(g)
            back_dve(NG - 1)
            if STAGE >= 4:
                nc.sync.dma_start(w0_d[:], w0all[:])
            back_gp(NG - 1)
            if STAGE < 2:
                nc.gpsimd.memset(w0all[:], 0.0)
                nc.sync.dma_start(w0_d[:], w0all[:])

    nc.compile()
    return nc


def _host_prep(x, Wq, bq, Wk, bk, Wv, bv):
    x = np.asarray(x, np.float32)
    Wq = np.asarray(Wq, np.float32)
    bq = np.asarray(bq, np.float32)
    Wk = np.asarray(Wk, np.float32)
    bk = np.asarray(bk, np.float32)
    Wv = np.asarray(Wv, np.float32)
    bv = np.asarray(bv, np.float32)

    # positional encoding (C, 9), matching reference._pos_encoding
    pos = np.arange(9, dtype=np.float32)[:, None]
    div = np.exp(np.arange(0, C, 2, dtype=np.float32) * (-np.log(10000.0) / C))
    pe = np.zeros((9, C), np.float32)
    pe[:, 0::2] = np.sin(pos * div)
    pe[:, 1::2] = np.cos(pos * div)
    pe = pe.T  # (C, 9)

    import ml_dtypes
    bf16 = ml_dtypes.bfloat16
    f8 = (ml_dtypes.float8_e4m3fn if hasattr(ml_dtypes, "float8_e4m3fn")
          else ml_dtypes.float8_e4m3)

    kp = Wk @ pe + bk[:, None]               # (A, 9), p = 0..8
    u9 = SCALE * (Wq.T @ kp)                 # (C, 9)
    cp = SCALE * (bq @ kp)                   # (9,)
    rr = SCALE * (Wk.T @ bq)                 # (C,)
    wm = np.concatenate([Wv.T, SCALE * (Wk.T @ Wq)], axis=1)

    wpk = np.zeros((C, 560), np.float32)
    wpk[:, 0:512] = wm
    wpk[:, 512:521] = u9
    wpk[:, 521] = rr
    wpk[0, 522:522 + GS * 9] = np.tile(cp, GS)
    wpk[:128, 558] = 1.0                     # w = W-1 mask
    wpk[63, 558] = 0.0
    wpk[127, 558] = 0.0

    wpk2 = np.concatenate([wpk[0:128], wpk[128:256]], axis=1)
    common = {"wpk": np.ascontiguousarray(wpk2.astype(bf16))}

    in_maps = []
    for b in range(B):
        xc = x[b].reshape(C, HW)
        xcm = np.zeros((128, 2, XCOLS), bf16)
        xcm[:, 0, 1:1 + HW] = xc[0:128]
        xcm[:, 1, 1:1 + HW] = xc[128:256]
        xpm = np.ascontiguousarray(
            xc.reshape(C, NT, 128).transpose(2, 1, 0).astype(f8)
        )
        in_maps.append({"xcm": xcm, "xpm": xpm, **common})
    return in_maps


def _host_post(results, bv):
    bv = np.asarray(bv, np.float32)
    out = np.empty((B, C, H, W), np.float32)
    for b in range(B):
        o_pm = results[b]["out"].astype(np.float32)       # [128, NT, C]
        w0 = results[b]["w0o"].astype(np.float32)         # [128, NT]
        o_pm += bv[None, None, :] * w0[:, :, None]
        out[b] = o_pm.transpose(2, 1, 0).reshape(C, H, W)
    return out


def kernel(x, Wq, bq, Wk, bk, Wv, bv):
    global LAST_RESULTS
    if "nc" not in _CACHE:
        _CACHE["nc"] = _build()
    nc = _CACHE["nc"]

    in_maps = _host_prep(x, Wq, bq, Wk, bk, Wv, bv)
    res = bass_utils.run_bass_kernel_spmd(
        nc, in_maps, core_ids=list(range(NCORES)),
        trace=bool(os.environ.get("KERNEL_TRACE")),
    )
    LAST_RESULTS = res
    return _host_post(res.results, bv)



# revision 2
# speedup vs baseline: 1.0134x; 1.0134x over previous
"""Trainium2 Bass kernel for nn_AttentionConv2D (sparse_attention) — v5.

Math (pos-never-incremented reference quirk): per pixel i (flat h*64+w):
    att0(i) = x_i^T G x_s(i) + u0^T x_i + r^T x_s(i) + cp0      G = s Wq^T Wk
    a_p(i)  = u_p^T x_i + cp_p                                  p = 1..8
    w0      = softmax([att0, a_1..a_8])[0]   (zeroed at w=W-1 / h=H-1)
    out     = (Wv x_s(i) + bv) * w0          x_s(i) = x at pixel i+65 (0-pad)

v5 design — fully decoupled engine loops (no cross-engine round trip inside
any single engine FIFO period):
  PE : per tile one fused [V|y] MM pair (N=512) + r/u smalls + cp-mask MM
  DVE: dots (STT from PSUM, ~466ns) + d4 + recip + s3 t0-add
  ACT: 4 PLAIN V copies PSUM->SBUF (never wait w0) + exp
  gp : w0 = ex0*rd4 (mask pre-folded into cp logits as -30) + 2 in-place
       pairwise V*w0 mults on out_sb (SBUF), 1 group behind
  w=W-1 mask: cp matmul uses a [2,128] stationary (ones; -30 mask row) so
  masked pixels get logit -30 -> e^0 ~ 0 -> w0 = 0.
PSUM: 5 banks [V|y] + 3 banks logits.
"""

import os
import sys

import numpy as np

for _p in ("/opt/trn_rl_repo",):
    if _p not in sys.path:
        sys.path.append(_p)

import concourse.bass as bass
import concourse.tile as tile
from concourse import bacc, mybir
from concourse import bass_utils

F32 = mybir.dt.float32
BF16 = mybir.dt.bfloat16
F8 = mybir.dt.float8e4
AF = mybir.ActivationFunctionType
ALU = mybir.AluOpType
AX = mybir.AxisListType

B, C, H, W = 8, 256, 64, 64
HW = H * W                # 4096
A = 256
SCALE = A ** -0.5
SHIFT = W + 1             # 65
NT = 32                   # pixel tiles (128 px each)
GS = 4                    # tiles per softmax group
NG = NT // GS             # 8 groups
XCOLS = HW + 68           # padded c-major x columns
NCORES = 8
WARMUP = int(os.environ.get("KERNEL_WARMUP", "7"))

_CACHE = {}
LAST_RESULTS = None


def _build():
    nc = bacc.Bacc("TRN2", target_bir_lowering=False, debug=False)

    xcm_d = nc.dram_tensor("xcm", [128, 2, XCOLS], BF16, kind="ExternalInput").ap()
    xpm_d = nc.dram_tensor("xpm", [128, NT, C], F8, kind="ExternalInput").ap()
    # packed constants per c-chunk k (cols k*624): 0:512 wm, 512:521 u9,
    # 521:522 rr, 522:558 cp36 (row0) / slot0-indicator (row1), 560:624 pad;
    # col block 1248:1376: row0 ones, row1 maskneg (-30 at w=63 partitions)
    wpk_d = nc.dram_tensor("wpk", [128, 1376], BF16, kind="ExternalInput").ap()
    out_d = nc.dram_tensor("out", [128, NT, C], BF16, kind="ExternalOutput").ap()
    w0_d = nc.dram_tensor("w0o", [128, NT], F32, kind="ExternalOutput").ap()

    with tile.TileContext(nc) as tc:
        with (
            tc.tile_pool(name="const", bufs=1) as const,
            tc.tile_pool(name="grp", bufs=4) as grp,
            tc.tile_pool(name="outp", bufs=4) as outp,
            tc.tile_pool(name="psVY", bufs=5, space="PSUM") as psVY,
            tc.tile_pool(name="psS", bufs=3, space="PSUM") as psS,
        ):
            # ---- persistent inputs ----
            xcm2_sb = const.tile([128, 2, XCOLS], BF16, name="xcm2", tag="xcm2")
            xcm_sb = [xcm2_sb[:, k, :] for k in range(2)]
            xpm_sb = const.tile([128, NT, C], F8, name="xpm", tag="xpm")
            wpk2_sb = const.tile([128, 1376], BF16, name="wpk2", tag="wpk2")
            wpk_sb = [wpk2_sb[:, k * 624:(k + 1) * 624] for k in range(2)]
            wm_sb = [wpk_sb[k][:, 0:512] for k in range(2)]
            u_sb = [wpk_sb[k][:, 512:521] for k in range(2)]
            r_sb = [wpk_sb[k][:, 521:522] for k in range(2)]
            cp2_sb = wpk_sb[0][0:2, 522:522 + GS * 9]   # row0 cp, row1 ind0
            onem_sb = wpk2_sb[0:2, 1248:1376]           # row0 ones, row1 maskneg
            w0all = const.tile([128, NT], F32, name="w0all", tag="w0all")
            t0all = const.tile([128, NT], F32, name="t0all", tag="t0all")
            scr = const.tile([128, C], BF16, name="scr", tag="scr")

            # gpsimd queue: warmup memset FIRST
            wu_sb = const.tile([128, 128], BF16, name="wu_sb", tag="wu_sb")
            nc.gpsimd.memset(wu_sb[:], 0.0)
            zro_sb = const.tile([128, 1], F32, name="zro", tag="zro")
            nc.vector.memset(zro_sb[:], 0.0)
            # warm the ACT exp table before the pipeline needs it
            nc.scalar.activation(scr[0:1, 0:1], zro_sb[0:1, :], AF.Exp,
                                 bias=zro_sb[0:1, :])

            # ---- input streaming: ONE ring (sync), strict need-order ----
            nc.sync.dma_start(wpk2_sb[:, 0:624], wpk_d[:, 0:624])
            nc.sync.dma_start(xcm2_sb[:, :, 0:706], xcm_d[:, :, 0:706])
            nc.sync.dma_start(wpk2_sb[:, 624:1376], wpk_d[:, 624:1376])
            nc.sync.dma_start(xpm_sb[:, 0:4, :], xpm_d[:, 0:4, :])
            nc.sync.dma_start(xcm2_sb[:, :, 706:1730], xcm_d[:, :, 706:1730])
            nc.sync.dma_start(xpm_sb[:, 4:12, :], xpm_d[:, 4:12, :])
            nc.sync.dma_start(xcm2_sb[:, :, 1730:2754], xcm_d[:, :, 1730:2754])
            nc.sync.dma_start(xpm_sb[:, 12:20, :], xpm_d[:, 12:20, :])
            nc.sync.dma_start(xcm2_sb[:, :, 2754:XCOLS], xcm_d[:, :, 2754:XCOLS])
            nc.sync.dma_start(xpm_sb[:, 20:32, :], xpm_d[:, 20:32, :])

            # ---- PE warm-up: matmuls on memset data, independent of DMAs ----
            if WARMUP:
                wu_ps = psVY.tile([128, 512], F32, name="wu", tag="vy")
                nc.tensor.matmul(
                    wu_ps[:, 0:16], wu_sb[:], wu_sb[:, 0:16],
                    start=True, stop=True,
                )
                nc.vector.tensor_scalar_add(scr[0:1, 0:1], wu_ps[0:1, 0:1], 0.0)
                for i in range(WARMUP - 1):
                    nc.tensor.matmul(
                        wu_ps[:, 128:256], wu_sb[:], wu_sb[:],
                        start=True, stop=True,
                    )

            state = {}
            vys = {}

            def front_tiles(g, jlist):
                s3 = state.get(("s3", g))
                if s3 is None:
                    s3 = psS.tile([128, GS * 9], F32, name=f"s{g}", tag="s")
                    state[("s3", g)] = s3
                for j in jlist:
                    t = g * GS + j
                    p0 = t * 128
                    vy = psVY.tile([128, 512], F32, name=f"vy{t}", tag="vy")
                    vys[t] = vy
                    xt = [xcm_sb[k][:, 1 + p0:1 + p0 + 128] for k in range(2)]
                    xs = [xcm_sb[k][:, 1 + p0 + SHIFT:1 + p0 + SHIFT + 128]
                          for k in range(2)]
                    if j == 0:
                        # cp broadcast + per-partition -30 mask on slot-0 cols
                        nc.tensor.matmul(
                            s3[:, 0:GS * 9], onem_sb, cp2_sb,
                            start=True, stop=False, skip_group_check=True,
                        )
                    # fused [V | y] GEMM + r logit, per c-chunk
                    nc.tensor.matmul(vy[:], xs[0], wm_sb[0], start=True,
                                     stop=False, skip_group_check=True)
                    nc.tensor.matmul(
                        s3[:, j * 9:j * 9 + 1], xs[0], r_sb[0],
                        start=False, stop=False, skip_group_check=True,
                    )
                    nc.tensor.matmul(vy[:], xs[1], wm_sb[1], start=False,
                                     stop=True, skip_group_check=True)
                    nc.tensor.matmul(
                        s3[:, j * 9:j * 9 + 1], xs[1], r_sb[1],
                        start=False, stop=False, skip_group_check=True,
                    )
                    # u logits
                    nc.tensor.matmul(
                        s3[:, j * 9:(j + 1) * 9], xt[0], u_sb[0],
                        start=False, stop=False, skip_group_check=True,
                    )
                    nc.tensor.matmul(
                        s3[:, j * 9:(j + 1) * 9], xt[1], u_sb[1],
                        start=False, stop=(j == GS - 1),
                        skip_group_check=True,
                    )
                    # att0 channel dot: t0all[:, t] = sum_c xpm * y
                    nc.vector.scalar_tensor_tensor(
                        scr[:], vy[:, 256:512], 1.0, xpm_sb[:, t, :],
                        ALU.mult, ALU.mult, accum_out=t0all[:, t:t + 1],
                    )

            def front_close(g):
                # s3 slot-0 += t0 for the whole group (DVE)
                s3 = state[("s3", g)]
                nc.vector.tensor_tensor(
                    s3[:, 0:GS * 9:9], s3[:, 0:GS * 9:9],
                    t0all[:, g * GS:(g + 1) * GS], ALU.add,
                )

            def mid(g):
                # ACT: plain V copies (no w0 dependency; frees PSUM fast),
                # then exp
                s3 = state.pop(("s3", g))
                out_sb = outp.tile([128, GS, C], BF16, name=f"o{g}", tag="o")
                for j in range(GS):
                    t = g * GS + j
                    nc.scalar.activation(
                        out_sb[:, j, :], vys.pop(t)[:, 0:256], AF.Identity,
                    )
                ex = grp.tile([128, GS, 9], BF16, name=f"ex{g}", tag="ex")
                nc.scalar.activation(ex[:], s3[:, 0:GS * 9], AF.Exp)
                state[g] = (ex, out_sb)

            def back_dve(g):
                # softmax scalar chain (deps resolved ~a group ago)
                ex, out_sb = state[g]
                d4 = grp.tile([128, GS], F32, name=f"d4{g}", tag="d4")
                nc.vector.tensor_reduce(d4[:], ex[:], axis=AX.X, op=ALU.add)
                rd4 = grp.tile([128, GS], F32, name=f"rd4{g}", tag="rd4")
                nc.vector.reciprocal_approx_fast(rd4[:], d4[:])
                # w0 = e0 * (1/D); mask already in the logits
                nc.gpsimd.tensor_tensor(
                    w0all[:, g * GS:(g + 1) * GS], ex[:, :, 0], rd4[:],
                    ALU.mult)
                if g == NG - 1:
                    # h = H-1 boundary: zero the last 64 pixels (tile 31)
                    nc.gpsimd.memset(w0all[64:128, NT - 1:NT], 0.0)

            def back_out(g):
                # V *= w0 in place on out_sb, pairwise; gp steady-state,
                # DVE for the final group (shorter tail)
                ex, out_sb = state.pop(g)
                last = g == NG - 1
                for jp in range(GS // 2):
                    t = g * GS + 2 * jp
                    dst = out_sb[:, 2 * jp:2 * jp + 2, :]
                    w0b = w0all[:, t:t + 2].broadcast_to((128, 2, C))
                    if last:
                        nc.vector.tensor_tensor(dst, dst, w0b, ALU.mult)
                    else:
                        nc.gpsimd.tensor_tensor(dst, dst, w0b, ALU.mult)
                if last:
                    nc.sync.dma_start(
                        out_d[:, g * GS:g * GS + 2, :], out_sb[:, 0:2, :])
                    nc.sync.dma_start(
                        out_d[:, g * GS + 2:(g + 1) * GS, :], out_sb[:, 2:4, :])
                else:
                    nc.sync.dma_start(
                        out_d[:, g * GS:(g + 1) * GS, :], out_sb[:])

            for g in range(NG):
                front_tiles(g, [0, 1])
                if g >= 1:
                    back_dve(g - 1)
                front_tiles(g, [2, 3])
                front_close(g)
                if g >= 1:
                    back_out(g - 1)
                mid(g)
            back_dve(NG - 1)
            back_out(NG - 1)
            nc.sync.dma_start(w0_d[:], w0all[:])

    nc.compile()
    return nc


def _host_prep(x, Wq, bq, Wk, bk, Wv, bv):
    x = np.asarray(x, np.float32)
    Wq = np.asarray(Wq, np.float32)
    bq = np.asarray(bq, np.float32)
    Wk = np.asarray(Wk, np.float32)
    bk = np.asarray(bk, np.float32)
    Wv = np.asarray(Wv, np.float32)
    bv = np.asarray(bv, np.float32)

    # positional encoding (C, 9), matching reference._pos_encoding
    pos = np.arange(9, dtype=np.float32)[:, None]
    div = np.exp(np.arange(0, C, 2, dtype=np.float32) * (-np.log(10000.0) / C))
    pe = np.zeros((9, C), np.float32)
    pe[:, 0::2] = np.sin(pos * div)
    pe[:, 1::2] = np.cos(pos * div)
    pe = pe.T  # (C, 9)

    import ml_dtypes
    bf16 = ml_dtypes.bfloat16
    f8 = (ml_dtypes.float8_e4m3fn if hasattr(ml_dtypes, "float8_e4m3fn")
          else ml_dtypes.float8_e4m3)

    kp = Wk @ pe + bk[:, None]               # (A, 9), p = 0..8
    u9 = SCALE * (Wq.T @ kp)                 # (C, 9)
    cp = SCALE * (bq @ kp)                   # (9,)
    rr = SCALE * (Wk.T @ bq)                 # (C,)
    wm = np.concatenate([Wv.T, SCALE * (Wk.T @ Wq)], axis=1)

    wpk = np.zeros((C, 624), np.float32)
    wpk[:, 0:512] = wm
    wpk[:, 512:521] = u9
    wpk[:, 521] = rr
    wpk[0, 522:522 + GS * 9] = np.tile(cp, GS)
    ind0 = np.zeros(GS * 9, np.float32)
    ind0[0::9] = 1.0
    wpk[1, 522:522 + GS * 9] = ind0          # slot-0 indicator row

    wpk2 = np.concatenate([wpk[0:128], wpk[128:256]], axis=1)  # (128, 1248)
    onem = np.zeros((128, 128), np.float32)
    onem[0, :] = 1.0                         # ones row (partition 0)
    onem[1, 63] = -30.0                      # maskneg row: w=63 partitions
    onem[1, 127] = -30.0
    wpk2 = np.concatenate([wpk2, onem], axis=1)                # (128, 1376)
    common = {"wpk": np.ascontiguousarray(wpk2.astype(bf16))}

    in_maps = []
    for b in range(B):
        xc = x[b].reshape(C, HW)
        xcm = np.zeros((128, 2, XCOLS), bf16)
        xcm[:, 0, 1:1 + HW] = xc[0:128]
        xcm[:, 1, 1:1 + HW] = xc[128:256]
        xpm = np.ascontiguousarray(
            xc.reshape(C, NT, 128).transpose(2, 1, 0).astype(f8)
        )
        in_maps.append({"xcm": xcm, "xpm": xpm, **common})
    return in_maps


def _host_post(results, bv):
    bv = np.asarray(bv, np.float32)
    out = np.empty((B, C, H, W), np.float32)
    for b in range(B):
        o_pm = results[b]["out"].astype(np.float32)       # [128, NT, C]
        w0 = results[b]["w0o"].astype(np.float32)         # [128, NT]
        o_pm += bv[None, None, :] * w0[:, :, None]
        out[b] = o_pm.transpose(2, 1, 0).reshape(C, H, W)
    return out


def kernel(x, Wq, bq, Wk, bk, Wv, bv):
    global LAST_RESULTS
    if "nc" not in _CACHE:
        _CACHE["nc"] = _build()
    nc = _CACHE["nc"]

    in_maps = _host_prep(x, Wq, bq, Wk, bk, Wv, bv)
    res = bass_utils.run_bass_kernel_spmd(
        nc, in_maps, core_ids=list(range(NCORES)),
        trace=bool(os.environ.get("KERNEL_TRACE")),
    )
    LAST_RESULTS = res
    return _host_post(res.results, bv)


# revision 5
# speedup vs baseline: 1.0452x; 1.0314x over previous
"""Trainium2 Bass kernel for nn_AttentionConv2D (sparse_attention) — v5.

Math (pos-never-incremented reference quirk): per pixel i (flat h*64+w):
    att0(i) = x_i^T G x_s(i) + u0^T x_i + r^T x_s(i) + cp0      G = s Wq^T Wk
    a_p(i)  = u_p^T x_i + cp_p                                  p = 1..8
    w0      = softmax([att0, a_1..a_8])[0]   (zeroed at w=W-1 / h=H-1)
    out     = (Wv x_s(i) + bv) * w0          x_s(i) = x at pixel i+65 (0-pad)

v5 design — fully decoupled engine loops (no cross-engine round trip inside
any single engine FIFO period):
  PE : per tile one fused [V|y] MM pair (N=512) + r/u smalls + cp-mask MM
  DVE: dots (STT from PSUM, ~466ns) + d4 + recip + s3 t0-add
  ACT: 4 PLAIN V copies PSUM->SBUF (never wait w0) + exp
  gp : w0 = ex0*rd4 (mask pre-folded into cp logits as -30) + 2 in-place
       pairwise V*w0 mults on out_sb (SBUF), 1 group behind
  w=W-1 mask: cp matmul uses a [2,128] stationary (ones; -30 mask row) so
  masked pixels get logit -30 -> e^0 ~ 0 -> w0 = 0.
PSUM: 5 banks [V|y] + 3 banks logits.
"""

import os
import sys

import numpy as np

for _p in ("/opt/trn_rl_repo",):
    if _p not in sys.path:
        sys.path.append(_p)

import concourse.bass as bass
import concourse.tile as tile
from concourse import bacc, mybir
from concourse import bass_utils

F32 = mybir.dt.float32
BF16 = mybir.dt.bfloat16
F8 = mybir.dt.float8e4
AF = mybir.ActivationFunctionType
ALU = mybir.AluOpType
AX = mybir.AxisListType

B, C, H, W = 8, 256, 64, 64
HW = H * W                # 4096
A = 256
SCALE = A ** -0.5
SHIFT = W + 1             # 65
NT = 32                   # pixel tiles (128 px each)
GS = 4                    # tiles per softmax group
NG = NT // GS             # 8 groups
XCOLS = HW + 68           # padded c-major x columns
NCORES = 8
WARMUP = int(os.environ.get("KERNEL_WARMUP", "9"))

_CACHE = {}
LAST_RESULTS = None


def _build():
    nc = bacc.Bacc("TRN2", target_bir_lowering=False, debug=False)

    xcm_d = nc.dram_tensor("xcm", [128, 2, XCOLS], BF16, kind="ExternalInput").ap()
    xpm_d = nc.dram_tensor("xpm", [128, NT, C], F8, kind="ExternalInput").ap()
    # packed constants per c-chunk k (cols k*624): 0:512 wm, 512:521 u9,
    # 521:522 rr, 522:558 cp36 (row0) / slot0-indicator (row1), 560:624 pad;
    # col block 1248:1376: row0 ones, row1 maskneg (-30 at w=63 partitions)
    wpk_d = nc.dram_tensor("wpk", [128, 1376], BF16, kind="ExternalInput").ap()
    out_d = nc.dram_tensor("out", [128, NT, C], BF16, kind="ExternalOutput").ap()
    w0_d = nc.dram_tensor("w0o", [128, NT], F32, kind="ExternalOutput").ap()

    with tile.TileContext(nc) as tc:
        with (
            tc.tile_pool(name="const", bufs=1) as const,
            tc.tile_pool(name="grp", bufs=4) as grp,
            tc.tile_pool(name="outp", bufs=4) as outp,
            tc.tile_pool(name="psVY", bufs=5, space="PSUM") as psVY,
            tc.tile_pool(name="psS", bufs=3, space="PSUM") as psS,
        ):
            # ---- persistent inputs ----
            xcm2_sb = const.tile([128, 2, XCOLS], BF16, name="xcm2", tag="xcm2")
            xcm_sb = [xcm2_sb[:, k, :] for k in range(2)]
            xpm_sb = const.tile([128, NT, C], F8, name="xpm", tag="xpm")
            wpk2_sb = const.tile([128, 1376], BF16, name="wpk2", tag="wpk2")
            wpk_sb = [wpk2_sb[:, k * 624:(k + 1) * 624] for k in range(2)]
            wm_sb = [wpk_sb[k][:, 0:512] for k in range(2)]
            u_sb = [wpk_sb[k][:, 512:521] for k in range(2)]
            r_sb = [wpk_sb[k][:, 521:522] for k in range(2)]
            cp2_sb = wpk_sb[0][0:2, 522:522 + GS * 9]   # row0 cp, row1 ind0
            onem_sb = wpk2_sb[0:2, 1248:1376]           # row0 ones, row1 maskneg
            w0all = const.tile([128, NT], F32, name="w0all", tag="w0all")
            t0all = const.tile([128, NT], F32, name="t0all", tag="t0all")
            scr = const.tile([128, C], BF16, name="scr", tag="scr")

            # gpsimd queue: warmup memset FIRST
            wu_sb = const.tile([128, 512], BF16, name="wu_sb", tag="wu_sb")
            nc.gpsimd.memset(wu_sb[:], 0.0)
            zro_sb = const.tile([128, 1], F32, name="zro", tag="zro")
            nc.vector.memset(zro_sb[:], 0.0)
            # warm the ACT exp table before the pipeline needs it
            nc.scalar.activation(scr[0:1, 0:1], zro_sb[0:1, :], AF.Exp,
                                 bias=zro_sb[0:1, :])

            # ---- input streaming: ONE ring (sync), strict need-order ----
            nc.sync.dma_start(wpk2_sb[:, 0:624], wpk_d[:, 0:624])
            nc.sync.dma_start(xcm2_sb[:, :, 0:706], xcm_d[:, :, 0:706])
            nc.sync.dma_start(wpk2_sb[:, 624:1376], wpk_d[:, 624:1376])
            nc.sync.dma_start(xpm_sb[:, 0:4, :], xpm_d[:, 0:4, :])
            nc.sync.dma_start(xcm2_sb[:, :, 706:1730], xcm_d[:, :, 706:1730])
            nc.sync.dma_start(xpm_sb[:, 4:12, :], xpm_d[:, 4:12, :])
            nc.sync.dma_start(xcm2_sb[:, :, 1730:2754], xcm_d[:, :, 1730:2754])
            nc.sync.dma_start(xpm_sb[:, 12:20, :], xpm_d[:, 12:20, :])
            nc.sync.dma_start(xcm2_sb[:, :, 2754:XCOLS], xcm_d[:, :, 2754:XCOLS])
            nc.sync.dma_start(xpm_sb[:, 20:32, :], xpm_d[:, 20:32, :])

            # ---- PE warm-up: matmuls on memset data, independent of DMAs ----
            if WARMUP:
                wu_ps = psVY.tile([128, 512], F32, name="wu", tag="vy")
                nc.tensor.matmul(
                    wu_ps[:, 0:16], wu_sb[:, 0:128], wu_sb[:, 0:16],
                    start=True, stop=True,
                )
                nc.vector.tensor_scalar_add(scr[0:1, 0:1], wu_ps[0:1, 0:1], 0.0)
                for i in range(WARMUP - 1):
                    nc.tensor.matmul(
                        wu_ps[:], wu_sb[:, 0:128], wu_sb[:],
                        start=True, stop=True,
                    )

            state = {}
            vys = {}

            def front_tiles(g, jlist):
                s3 = state.get(("s3", g))
                if s3 is None:
                    s3 = psS.tile([128, GS * 9], F32, name=f"s{g}", tag="s")
                    state[("s3", g)] = s3
                for j in jlist:
                    t = g * GS + j
                    p0 = t * 128
                    vy = psVY.tile([128, 512], F32, name=f"vy{t}", tag="vy")
                    vys[t] = vy
                    xt = [xcm_sb[k][:, 1 + p0:1 + p0 + 128] for k in range(2)]
                    xs = [xcm_sb[k][:, 1 + p0 + SHIFT:1 + p0 + SHIFT + 128]
                          for k in range(2)]
                    if j == 0:
                        # cp broadcast + per-partition -30 mask on slot-0 cols
                        nc.tensor.matmul(
                            s3[:, 0:GS * 9], onem_sb, cp2_sb,
                            start=True, stop=False, skip_group_check=True,
                        )
                    # fused [V | y] GEMM + r logit, per c-chunk
                    nc.tensor.matmul(vy[:], xs[0], wm_sb[0], start=True,
                                     stop=False, skip_group_check=True)
                    nc.tensor.matmul(
                        s3[:, j * 9:j * 9 + 1], xs[0], r_sb[0],
                        start=False, stop=False, skip_group_check=True,
                    )
                    nc.tensor.matmul(vy[:], xs[1], wm_sb[1], start=False,
                                     stop=True, skip_group_check=True)
                    nc.tensor.matmul(
                        s3[:, j * 9:j * 9 + 1], xs[1], r_sb[1],
                        start=False, stop=False, skip_group_check=True,
                    )
                    # u logits
                    nc.tensor.matmul(
                        s3[:, j * 9:(j + 1) * 9], xt[0], u_sb[0],
                        start=False, stop=False, skip_group_check=True,
                    )
                    nc.tensor.matmul(
                        s3[:, j * 9:(j + 1) * 9], xt[1], u_sb[1],
                        start=False, stop=(j == GS - 1),
                        skip_group_check=True,
                    )
                    # att0 channel dot: t0all[:, t] = sum_c xpm * y
                    nc.vector.scalar_tensor_tensor(
                        scr[:], vy[:, 256:512], 1.0, xpm_sb[:, t, :],
                        ALU.mult, ALU.mult, accum_out=t0all[:, t:t + 1],
                    )

            def front_close(g):
                # s3 slot-0 += t0 for the whole group (DVE)
                s3 = state[("s3", g)]
                nc.vector.tensor_tensor(
                    s3[:, 0:GS * 9:9], s3[:, 0:GS * 9:9],
                    t0all[:, g * GS:(g + 1) * GS], ALU.add,
                )

            def mid(g):
                # ACT: plain V copies (no w0 dependency; frees PSUM fast),
                # then exp
                s3 = state.pop(("s3", g))
                out_sb = outp.tile([128, GS, C], BF16, name=f"o{g}", tag="o")
                for j in range(GS):
                    t = g * GS + j
                    nc.scalar.activation(
                        out_sb[:, j, :], vys.pop(t)[:, 0:256], AF.Identity,
                    )
                ex = grp.tile([128, GS, 9], BF16, name=f"ex{g}", tag="ex")
                nc.scalar.activation(ex[:], s3[:, 0:GS * 9], AF.Exp)
                state[g] = (ex, out_sb)

            def back_dve(g):
                # softmax scalar chain (deps resolved ~a group ago)
                ex, out_sb = state[g]
                d4 = grp.tile([128, GS], F32, name=f"d4{g}", tag="d4")
                nc.vector.tensor_reduce(d4[:], ex[:], axis=AX.X, op=ALU.add)
                rd4 = grp.tile([128, GS], F32, name=f"rd4{g}", tag="rd4")
                nc.vector.reciprocal_approx_fast(rd4[:], d4[:])
                # w0 = e0 * (1/D); mask already in the logits
                nc.gpsimd.tensor_tensor(
                    w0all[:, g * GS:(g + 1) * GS], ex[:, :, 0], rd4[:],
                    ALU.mult)
                if g == NG - 1:
                    # h = H-1 boundary: zero the last 64 pixels (tile 31)
                    nc.gpsimd.memset(w0all[64:128, NT - 1:NT], 0.0)

            def back_out(g):
                # V *= w0 in place on out_sb, pairwise; gp steady-state,
                # DVE for the final group (shorter tail)
                ex, out_sb = state.pop(g)
                last = g == NG - 1
                for jp in range(GS // 2):
                    t = g * GS + 2 * jp
                    dst = out_sb[:, 2 * jp:2 * jp + 2, :]
                    w0b = w0all[:, t:t + 2].broadcast_to((128, 2, C))
                    if last:
                        nc.vector.tensor_tensor(dst, dst, w0b, ALU.mult)
                    else:
                        nc.gpsimd.tensor_tensor(dst, dst, w0b, ALU.mult)
                if last:
                    nc.sync.dma_start(
                        out_d[:, g * GS:g * GS + 2, :], out_sb[:, 0:2, :])
                    nc.sync.dma_start(
                        out_d[:, g * GS + 2:(g + 1) * GS, :], out_sb[:, 2:4, :])
                else:
                    nc.sync.dma_start(
                        out_d[:, g * GS:(g + 1) * GS, :], out_sb[:])

            for g in range(NG):
                front_tiles(g, [0, 1])
                if g >= 1:
                    back_dve(g - 1)
                front_tiles(g, [2, 3])
                front_close(g)
                if g >= 1:
                    back_out(g - 1)
                mid(g)
            back_dve(NG - 1)
            back_out(NG - 1)
            nc.sync.dma_start(w0_d[:], w0all[:])

    nc.compile()
    return nc


def _host_prep(x, Wq, bq, Wk, bk, Wv, bv):
    x = np.asarray(x, np.float32)
    Wq = np.asarray(Wq, np.float32)
    bq = np.asarray(bq, np.float32)
    Wk = np.asarray(Wk, np.float32)
    bk = np.asarray(bk, np.float32)
    Wv = np.asarray(Wv, np.float32)
    bv = np.asarray(bv, np.float32)

    # positional encoding (C, 9), matching reference._pos_encoding
    pos = np.arange(9, dtype=np.float32)[:, None]
    div = np.exp(np.arange(0, C, 2, dtype=np.float32) * (-np.log(10000.0) / C))
    pe = np.zeros((9, C), np.float32)
    pe[:, 0::2] = np.sin(pos * div)
    pe[:, 1::2] = np.cos(pos * div)
    pe = pe.T  # (C, 9)

    import ml_dtypes
    bf16 = ml_dtypes.bfloat16
    f8 = (ml_dtypes.float8_e4m3fn if hasattr(ml_dtypes, "float8_e4m3fn")
          else ml_dtypes.float8_e4m3)

    kp = Wk @ pe + bk[:, None]               # (A, 9), p = 0..8
    u9 = SCALE * (Wq.T @ kp)                 # (C, 9)
    cp = SCALE * (bq @ kp)                   # (9,)
    rr = SCALE * (Wk.T @ bq)                 # (C,)
    wm = np.concatenate([Wv.T, SCALE * (Wk.T @ Wq)], axis=1)

    wpk = np.zeros((C, 624), np.float32)
    wpk[:, 0:512] = wm
    wpk[:, 512:521] = u9
    wpk[:, 521] = rr
    wpk[0, 522:522 + GS * 9] = np.tile(cp, GS)
    ind0 = np.zeros(GS * 9, np.float32)
    ind0[0::9] = 1.0
    wpk[1, 522:522 + GS * 9] = ind0          # slot-0 indicator row

    wpk2 = np.concatenate([wpk[0:128], wpk[128:256]], axis=1)  # (128, 1248)
    onem = np.zeros((128, 128), np.float32)
    onem[0, :] = 1.0                         # ones row (partition 0)
    onem[1, 63] = -30.0                      # maskneg row: w=63 partitions
    onem[1, 127] = -30.0
    wpk2 = np.concatenate([wpk2, onem], axis=1)                # (128, 1376)
    common = {"wpk": np.ascontiguousarray(wpk2.astype(bf16))}

    in_maps = []
    for b in range(B):
        xc = x[b].reshape(C, HW)
        xcm = np.zeros((128, 2, XCOLS), bf16)
        xcm[:, 0, 1:1 + HW] = xc[0:128]
        xcm[:, 1, 1:1 + HW] = xc[128:256]
        xpm = np.ascontiguousarray(
            xc.reshape(C, NT, 128).transpose(2, 1, 0).astype(f8)
        )
        in_maps.append({"xcm": xcm, "xpm": xpm, **common})
    return in_maps


def _host_post(results, bv):
    bv = np.asarray(bv, np.float32)
    out = np.empty((B, C, H, W), np.float32)
    for b in range(B):
        o_pm = results[b]["out"].astype(np.float32)       # [128, NT, C]
        w0 = results[b]["w0o"].astype(np.float32)         # [128, NT]
        o_pm += bv[None, None, :] * w0[:, :, None]
        out[b] = o_pm.transpose(2, 1, 0).reshape(C, H, W)
    return out


def kernel(x, Wq, bq, Wk, bk, Wv, bv):
    global LAST_RESULTS
    if "nc" not in _CACHE:
        _CACHE["nc"] = _build()
    nc = _CACHE["nc"]

    in_maps = _host_prep(x, Wq, bq, Wk, bk, Wv, bv)
    res = bass_utils.run_bass_kernel_spmd(
        nc, in_maps, core_ids=list(range(NCORES)),
        trace=bool(os.environ.get("KERNEL_TRACE")),
    )
    LAST_RESULTS = res
    return _host_post(res.results, bv)
